# revision 1
# baseline (speedup 1.0000x reference)
"""Trainium2 Bass kernel: single-channel Conv2d.

  x: [32, 224, 224] f32, kernels: [64, 7, 7] f32
  out[b, k, i, j] = sum_{di,dj} x[b, i+di, j+dj] * kernels[k, di, dj]
  -> [32, 64, 218, 218]

Sharding: data-parallel over batch, 4 images per NeuronCore across 8 cores.

Per-core algorithm (fp32r matmuls: fp32 rounded to 11-bit mantissa, which
streams at full PE rate):
  - 4 images = 2 image-pairs. An image-pair's rows are staged in SBUF as
    x2s[row, seg*464 + img*224 + j] (two row-segments 0..127 / 120..223 with
    an 8-row halo, 448 data cols + zero pad per segment).
  - VectorE builds a shift-expanded fp32r copy
        x2g[row, seg*1824 + g*456 + c] = x2s[row, seg*464 + c + g], g=0..3
    (4 column-shifted copies along the free dim; also applies f32r rounding).
  - For each output-row-pair (i, i+1), ONE rectangular SBUF->SBUF DMA
    gathers the patch tile pt[32, 456]:
        pt[dr*4 + g, c] = x2g[i + dr, seg_off + g*456 + c]
    (out is a contiguous [32, 456] tile; in is a plain [8, 1824] slice).
  - Two accumulating matmuls (tap groups d=0,4) with banded 32x128
    stationary weights (precomputed on host, rounded to f32r on device)
    produce a full PSUM tile [128 = 2 rows x 64 ch, 448 = 2 imgs x 224]:
        W[d][dr*4+g, s*64+k] = w[k, dr-s, g+d]   (stream offset d applies
        taps dj = g+d; out-of-band entries are zero).
  - PSUM is evacuated by VectorE+ScalarE into a 16-pair SBUF chunk, which
    is stored with 4 large DMAs (s x img).
  - DMA issue is split between the SP (HWDGE) and Pool (SWDGE) queues.
"""
import sys

sys.path.insert(0, "/opt/trn_rl_repo")

import numpy as np

B, H, W = 32, 224, 224
KCH, KS = 64, 7
HO = WO = H - KS + 1  # 218
NCORES = 8
BLOC = B // NCORES    # 4 images per core
NPAIRS = HO // 2      # 109 output-row-pairs per image-pair

SEGW = 464            # x2s per-segment span (448 data + 16 zero pad)
X2SF = 2 * SEGW       # 928
GSP = 456             # x2g per-shift span (= pt free size)
NG = 4                # shift groups
X2GF = 2 * NG * GSP   # 3648
PTW = GSP             # 456
NST = 448             # matmul stream length (2 imgs x 224)
DVE_COLS = 280        # PSUM evacuation split: VectorE cols, rest ScalarE
CH = 16               # row-pairs per output SBUF chunk
OIMG = KCH * HO * WO

_NC_CACHE = {}


def make_weight_band(kernels: np.ndarray) -> np.ndarray:
    """Banded stationary matrices [2, 32, 128]: index dd covers taps
    dj = g + 4*dd.  W[dd][dr*4 + g, s*64 + k] = kernels[k, dr-s, g+4*dd]."""
    wb = np.zeros((2, 32, 128), dtype=np.float32)
    for dd in range(2):
        d = 4 * dd
        for dr in range(8):
            for g in range(NG):
                dj = g + d
                if dj > KS - 1:
                    continue
                p = dr * 4 + g
                for s in range(2):
                    di = dr - s
                    if 0 <= di < KS:
                        wb[dd, p, s * KCH: (s + 1) * KCH] = kernels[:, di, dj]
    return wb


def _build_nc(iters: int = 1, no_stores: bool = False, no_evac: bool = False,
              no_mm: bool = False, stores_only: bool = False, ch: int = CH):
    import concourse.bacc as bacc
    import concourse.mybir as mybir
    import concourse.tile as tile
    from concourse.bass_types import AP

    F32 = mybir.dt.float32
    F32R = mybir.dt.float32r

    nc = bacc.Bacc("TRN2", target_bir_lowering=False, debug=False,
                   num_devices=NCORES)
    x_d = nc.dram_tensor("x", [BLOC, H, W], F32, kind="ExternalInput").ap()
    wb_d = nc.dram_tensor("wband", [2, 32, 128], F32,
                          kind="ExternalInput").ap()
    out_d = nc.dram_tensor("out", [BLOC, KCH, HO, WO], F32,
                           kind="ExternalOutput").ap()

    with tile.TileContext(nc) as tc:
        with (
            tc.tile_pool(name="wpool", bufs=1) as wpool,
            tc.tile_pool(name="x2pool", bufs=2) as x2pool,
            tc.tile_pool(name="ptpool", bufs=8) as ptpool,
            tc.tile_pool(name="opool", bufs=3) as opool,
            tc.tile_pool(name="psum", bufs=8, space="PSUM") as psum,
        ):
            # ---- stationary weights: [32, 2*128] f32r ----
            wb32 = wpool.tile([32, 2 * 128], F32)
            nc.sync.dma_start(out=wb32[:],
                              in_=wb_d.rearrange("i p m -> p i m"))
            wbr = wpool.tile([32, 2 * 128], F32R)
            nc.vector.tensor_copy(out=wbr[:], in_=wb32[:])

            def body():
                for q in range(2):
                    x2s = x2pool.tile([128, X2SF], F32, tag="x2s")
                    nc.gpsimd.memset(x2s[:], 0.0)
                    for seg in range(2):
                        r_lo = 0 if seg == 0 else 120
                        nrows = 128 if seg == 0 else H - 120
                        nc.sync.dma_start(
                            out=x2s[0:nrows, seg * SEGW: seg * SEGW + 2 * W]
                            .rearrange("r (b j) -> r b j", b=2),
                            in_=x_d[2 * q: 2 * q + 2, r_lo: r_lo + nrows, :]
                            .rearrange("b r j -> r b j"),
                        )
                    # shift-expanded f32r copy
                    x2g = x2pool.tile([128, X2GF], F32R, tag="x2g")
                    for seg in range(2):
                        for g in range(NG):
                            nc.vector.tensor_copy(
                                out=x2g[:, (seg * NG + g) * GSP:
                                        (seg * NG + g + 1) * GSP],
                                in_=x2s[:, seg * SEGW + g:
                                        seg * SEGW + g + GSP],
                            )

                    chunk = None
                    npl = 0
                    chunk_start = 0
                    for pr in range(NPAIRS):
                        i = 2 * pr
                        if pr % ch == 0:
                            npl = min(ch, NPAIRS - pr)
                            chunk = opool.tile([128, ch * NST], F32,
                                               tag="osb")
                            chunk_start = pr
                        seg = 0 if i + 7 <= 127 else 1
                        r0 = i - 120 * seg
                        goff = seg * NG * GSP
                        pt = ptpool.tile([32, PTW], F32R, tag="pt")
                        if not stores_only:
                            dma_eng = nc.sync if pr % 2 == 0 else nc.scalar
                            dma_eng.dma_start(
                                out=pt[:],
                                in_=x2g[r0: r0 + 8, goff: goff + NG * GSP],
                            )
                        pl = pr - chunk_start
                        ps = psum.tile([128, NST], F32)
                        if not (no_mm or stores_only):
                            for dd in range(2):
                                d = 4 * dd
                                nc.tensor.matmul(
                                    out=ps[:],
                                    lhsT=wbr[:, dd * 128: (dd + 1) * 128],
                                    rhs=pt[:, d: d + NST],
                                    start=(dd == 0), stop=(dd == 1),
                                )
                        if not no_evac and not no_mm and not stores_only:
                            nc.vector.tensor_copy(
                                out=chunk[:, pl * NST: pl * NST + DVE_COLS],
                                in_=ps[:, 0:DVE_COLS])
                            nc.scalar.copy(
                                out=chunk[:, pl * NST + DVE_COLS:
                                          (pl + 1) * NST],
                                in_=ps[:, DVE_COLS:NST])
                        if no_stores or no_evac or no_mm:
                            continue
                        if pl == npl - 1:
                            F = ch * NST
                            st_engines = (nc.sync, nc.gpsimd)
                            nst = 0
                            for s in range(2):
                                for img in range(2):
                                    for kh in range(2):  # k-halves
                                        kw = KCH // 2
                                        st_in = AP(
                                            tensor=chunk[:].tensor,
                                            offset=chunk[:].offset
                                            + (s * KCH + kh * kw) * F
                                            + img * W,
                                            ap=((F, kw), (NST, npl),
                                                (1, WO)),
                                        )
                                        st_out = AP(
                                            tensor=out_d.tensor,
                                            offset=(2 * q + img) * OIMG
                                            + kh * kw * HO * WO
                                            + (2 * chunk_start + s) * WO,
                                            ap=((HO * WO, kw), (2 * WO, npl),
                                                (1, WO)),
                                        )
                                        st_engines[nst % 2].dma_start(
                                            out=st_out, in_=st_in)
                                        nst += 1

            if iters == 1:
                body()
            else:
                with tc.For_i(0, iters, 1):
                    body()
    nc.compile()
    return nc


def _get_nc(iters: int = 1, **kw):
    key = (iters, tuple(sorted(kw.items())))
    if key not in _NC_CACHE:
        _NC_CACHE[key] = _build_nc(iters, **kw)
    return _NC_CACHE[key]


def kernel(x: np.ndarray, kernels: np.ndarray) -> np.ndarray:
    from concourse.bass_utils import run_bass_kernel_spmd

    x = np.ascontiguousarray(np.asarray(x, dtype=np.float32))
    kernels = np.ascontiguousarray(np.asarray(kernels, dtype=np.float32))
    wb = make_weight_band(kernels)
    nc = _get_nc()
    in_maps = [
        {"x": x[c * BLOC: (c + 1) * BLOC], "wband": wb}
        for c in range(NCORES)
    ]
    res = run_bass_kernel_spmd(nc, in_maps, core_ids=list(range(NCORES)))
    return np.concatenate([res.results[c]["out"] for c in range(NCORES)],
                          axis=0)

